# revision 8
# baseline (speedup 1.0000x reference)
"""DeepSeek-MoE layer on 8 Trainium2 NeuronCores (expert-parallel, sparse dispatch).

Sharding:
  - Expert-parallel: core c owns experts [4c, 4c+4). The expert axis is
    permuted per core in the staged inputs so every core's LOCAL experts are
    logits columns 0..3 (keeps the SPMD program core-independent).
  - Shared expert: sharded on the intermediate dim (core c owns a 1024-wide
    slice of I=8192); partial [T, H] outputs summed on host.
  - Router + x replicated; router logits output taken from core 0.

Device program per core:
  1. Router logits [E, T] in true fp32 (top-4 selection is gap-sensitive:
     min 4th/5th logit gap in-distribution is ~5e-5; fp32r's ~5e-4 error
     would flip selections).
  2. Per 128-token tile: DVE max8 -> top-4 threshold + softmax combine
     weights cw (matches jax top_k + softmax for distinct logits).
  3. Per local expert: GPSIMD sparse_gather compacts routed token ids
     (capacity C=384 >= observed max 281, tail-padded with out-of-range ids
     so padded slots self-identify); ap_gather pulls the tokens' hidden
     vectors from the resident x; bounced via DRAM to free SBUF.
  4. Shared-expert FFN over all tokens for the local I-slice.
  5. Expert FFNs over gathered tokens only (fp32r matmuls, silu on ACT,
     combine-weight scaling on device); slot outputs DMA'd out.

Host: scatter-add slot outputs to token rows (the all-to-all return), sum
per-core shared partials, unpermute core 0's logits.
"""

import numpy as np

import concourse.bacc as bacc
import concourse.mybir as mybir
import concourse.tile as tile
from concourse.bass_utils import run_bass_kernel_spmd
from concourse.masks import make_identity

F32 = mybir.dt.float32
F32R = mybir.dt.float32r
I16 = mybir.dt.int16
U32 = mybir.dt.uint32
AF = mybir.ActivationFunctionType
ALU = mybir.AluOpType
AXX = mybir.AxisListType.X

N_CORES = 8
B, S, H, E, K, M, I = 2, 1024, 2048, 32, 4, 1408, 8192
T = B * S                      # 2048 tokens
HB = H // 128                  # 16 h blocks
EL = E // N_CORES              # 4 local experts
MT = M // 128                  # 11 m tiles
ISH = I // N_CORES             # 1024 shared-intermediate slice
IT = ISH // 128                # 8 i tiles
C = 384                        # per-expert token capacity (max observed 281)
CW = C // 16
TW = T // 16                   # 128
NCH = 4                        # 512-token chunks
MGROUPS = ((0, 4), (4, 8), (8, 11))

_CACHE = {}


def _rt(x):
    return x.bitcast(F32R)


def _build_nc():
    nc = bacc.Bacc("TRN2", target_bir_lowering=False, debug=False,
                   num_devices=N_CORES)

    xtb = nc.dram_tensor("xtb", [128, T, HB], F32, kind="ExternalInput")
    xtbr = nc.dram_tensor("xtbr", [HB, 128, T], F32R, kind="ExternalInput")
    gwb = nc.dram_tensor("gwb", [128, HB, E], F32, kind="ExternalInput")
    w1l = nc.dram_tensor("w1l", [EL, HB, 128, M], F32R, kind="ExternalInput")
    w2l = nc.dram_tensor("w2l", [EL, MT, 128, H], F32R, kind="ExternalInput")
    ws1c = nc.dram_tensor("ws1c", [HB, 128, ISH], F32R, kind="ExternalInput")
    ws2c = nc.dram_tensor("ws2c", [IT, 128, H], F32R, kind="ExternalInput")
    iotap1 = nc.dram_tensor("iotap1", [16, TW + CW], F32, kind="ExternalInput")

    logits_out = nc.dram_tensor("logits_out", [T, E], F32, kind="ExternalOutput")
    shared_out = nc.dram_tensor("shared_out", [T, H], F32, kind="ExternalOutput")
    slots_out = nc.dram_tensor("slots_out", [EL, C, H], F32, kind="ExternalOutput")
    tok_out = nc.dram_tensor("tok_out", [EL, 16, CW], F32, kind="ExternalOutput")

    cwT_dram = nc.dram_tensor("cwT_dram", [E, T], F32)
    xe_dram = nc.dram_tensor("xe_dram", [EL, 128, HB, C], F32R)

    with tile.TileContext(nc) as tc:
        with tc.tile_pool(name="pp", bufs=1) as pp:
            ident = pp.tile([128, 128], F32)
            make_identity(nc, ident[:])
            iop1 = pp.tile([16, TW + CW], F32)
            nc.sync.dma_start(iop1[:], iotap1[:])
            cwga_keep = []
            idx_keep = []

            with tc.tile_pool(name="xp", bufs=1) as xp:
                x_sb = xp.tile([128, T, HB], F32)
                for ch in range(16):
                    nc.sync.dma_start(x_sb[:, ch * 128:(ch + 1) * 128, :],
                                      xtb[:, ch * 128:(ch + 1) * 128, :])

                # ============ router + cw (true fp32) ============
                with tc.tile_pool(name="rp", bufs=1) as rp, \
                     tc.tile_pool(name="rw", bufs=2) as rw, \
                     tc.tile_pool(name="ps1", bufs=1, space="PSUM") as ps1:
                    g_sb = rp.tile([128, HB, E], F32)
                    nc.sync.dma_start(g_sb[:], gwb[:])
                    lsb = rp.tile([E, T], F32)
                    for ch in range(NCH):
                        pt = ps1.tile([E, 512], F32, tag="rps")
                        for hb in range(HB):
                            nc.tensor.matmul(
                                pt[:], lhsT=g_sb[:, hb, :],
                                rhs=x_sb[:, ch * 512:(ch + 1) * 512, hb],
                                start=(hb == 0), stop=(hb == HB - 1))
                        nc.vector.tensor_copy(lsb[:, ch * 512:(ch + 1) * 512],
                                              pt[:])

                    cwT = rp.tile([E, T], F32)
                    for j in range(T // 128):
                        ptt = ps1.tile([128, E], F32, tag="tps")
                        nc.tensor.transpose(
                            ptt[:], lsb[:, j * 128:(j + 1) * 128], ident[:E, :E])
                        lf = rw.tile([128, E], F32, tag="lf")
                        nc.vector.tensor_copy(lf[:], ptt[:])
                        nc.sync.dma_start(logits_out[j * 128:(j + 1) * 128, :],
                                          lf[:])

                        m8 = rw.tile([128, 8], F32, tag="m8")
                        nc.vector.max(out=m8[:], in_=lf[:])
                        negmax = rw.tile([128, 1], F32, tag="negmax")
                        nc.vector.tensor_scalar_mul(negmax[:], m8[:, 0:1], -1.0)
                        exp4 = rw.tile([128, 4], F32, tag="exp4")
                        nc.scalar.activation(exp4[:], m8[:, 0:4], AF.Exp,
                                             bias=negmax[:])
                        den = rw.tile([128, 1], F32, tag="den")
                        nc.vector.reduce_sum(den[:], exp4[:], axis=AXX)
                        rden = rw.tile([128, 1], F32, tag="rden")
                        nc.vector.reciprocal(rden[:], den[:])

                        mask = rw.tile([128, E], F32, tag="mask")
                        nc.vector.tensor_tensor(
                            mask[:], lf[:], m8[:, 3:4].to_broadcast([128, E]),
                            op=ALU.is_ge)
                        cwt = rw.tile([128, E], F32, tag="cwt")
                        nc.scalar.activation(cwt[:], lf[:], AF.Exp,
                                             bias=negmax[:])
                        nc.vector.tensor_mul(cwt[:], cwt[:], mask[:])
                        nc.vector.tensor_scalar_mul(cwt[:], cwt[:], rden[:])

                        ptc = ps1.tile([E, 128], F32, tag="tpsb")
                        nc.tensor.transpose(ptc[:], cwt[:], ident[:])
                        nc.vector.tensor_copy(cwT[:, j * 128:(j + 1) * 128],
                                              ptc[:])
                    nc.sync.dma_start(cwT_dram[:], cwT[:])

                # ============ dispatch: tables + token gathers ============
                with tc.tile_pool(name="dw", bufs=2) as dw, \
                     tc.tile_pool(name="db", bufs=1) as db, \
                     tc.tile_pool(name="ps2", bufs=2, space="PSUM") as ps2:
                    for e in range(EL):
                        v = dw.tile([128, 16], F32, tag="vrow")
                        nc.sync.dma_start(
                            v[:], cwT_dram[e].rearrange("(f q) -> f q", q=16))
                        wps = ps2.tile([16, 128], F32, tag="wps")
                        nc.tensor.transpose(wps[:], v[:], ident[:])
                        wext = dw.tile([16, TW + CW], F32, tag="wext")
                        nc.vector.tensor_copy(wext[:, :TW], wps[:])
                        nc.vector.memset(wext[:, TW:], 1.0)

                        ge0 = dw.tile([16, TW + CW], F32, tag="ge0")
                        nc.vector.tensor_scalar(ge0[:], wext[:], 0.0, None,
                                                op0=ALU.is_gt)
                        sgt = dw.tile([16, TW + CW], F32, tag="sgt")
                        nc.vector.tensor_mul(sgt[:], ge0[:], iop1[:])
                        nc.vector.tensor_scalar_add(sgt[:], sgt[:], -1.0)

                        tok_sl = dw.tile([16, CW], F32, tag="toksl")
                        nfound = dw.tile([1, 1], U32, tag="nf")
                        nc.gpsimd.sparse_gather(tok_sl[:], sgt[:],
                                                num_found=nfound[:])
                        nc.sync.dma_start(tok_out[e], tok_sl[:])

                        tok_cl = dw.tile([16, CW], F32, tag="tokcl")
                        nc.vector.tensor_scalar_min(tok_cl[:], tok_sl[:],
                                                    float(T - 1))
                        tok_i16 = dw.tile([16, CW], I16, tag="toki")
                        nc.vector.tensor_copy(tok_i16[:], tok_cl[:])
                        idx128 = pp.tile([128, CW], I16, tag=f"idx{e}")
                        for k in range(8):
                            nc.sync.dma_start(idx128[k * 16:(k + 1) * 16, :],
                                              tok_i16[:])
                        idx_keep.append(idx128)

                        cwrep = db.tile([128, T], F32, tag="cwrep")
                        nc.sync.dma_start(cwrep[0:1, :], cwT_dram[e][None, :])
                        p = 1
                        while p < 128:
                            nc.sync.dma_start(cwrep[p:2 * p, :], cwrep[0:p, :])
                            p *= 2
                        cwga = pp.tile([128, C], F32, tag=f"cwga{e}")
                        nc.gpsimd.ap_gather(cwga[:], cwrep[:], idx128[:],
                                            channels=128, num_elems=T, d=1,
                                            num_idxs=C)
                        cwga_keep.append(cwga)

                        xrp = db.tile([128, HB, C], F32, tag="xrp")
                        for g in range(4):
                            raw = db.tile([128, 96, HB], F32, tag="raw")
                            nc.gpsimd.ap_gather(
                                raw[:], x_sb[:], idx128[:, 6 * g:6 * g + 6],
                                channels=128, num_elems=T, d=HB, num_idxs=96)
                            for hb in range(HB):
                                nc.vector.tensor_copy(
                                    xrp[:, hb, 96 * g:96 * (g + 1)],
                                    raw[:, :, hb])
                        nc.gpsimd.dma_start(xe_dram[e], xrp[:])

                # ============ shared expert (I-slice) ============
            # (x streamed from DRAM in h-major fp32r; x_sb no longer needed)
            with tc.tile_pool(name="shp", bufs=2) as shp, \
                 tc.tile_pool(name="shb", bufs=1) as shb:
                for half in range(2):
                    t0 = half * 1024
                    s_sb = shb.tile([128, IT, 1024], F32R, tag="s_sb")
                    with tc.tile_pool(name="psu", bufs=1,
                                      space="PSUM") as psu:
                        for g0 in (0, 4):
                            pts = [psu.tile([128, 512], F32,
                                            tag=f"su{q}", name=f"su{q}")
                                   for q in range(8)]
                            for hb in range(HB):
                                xrh = shp.tile([128, 1024], F32R, tag="xrh")
                                nc.sync.dma_start(
                                    xrh[:], xtbr[hb, :, t0:t0 + 1024])
                                wt = shp.tile([128, 512], F32R, tag="ws1t")
                                nc.sync.dma_start(
                                    wt[:],
                                    ws1c[hb, :, g0 * 128:(g0 + 4) * 128])
                                for q in range(8):
                                    it, c2 = g0 + q // 2, q % 2
                                    nc.tensor.matmul(
                                        pts[q][:],
                                        lhsT=wt[:, (it - g0) * 128:
                                                (it - g0 + 1) * 128],
                                        rhs=xrh[:, c2 * 512:(c2 + 1) * 512],
                                        start=(hb == 0), stop=(hb == HB - 1))
                            for q in range(8):
                                it, c2 = g0 + q // 2, q % 2
                                nc.scalar.activation(
                                    s_sb[:, it, c2 * 512:(c2 + 1) * 512],
                                    pts[q][:], AF.Silu)
                    with tc.tile_pool(name="psd", bufs=1,
                                      space="PSUM") as psd:
                        for hs in range(4):
                            pods = [psd.tile([128, 512], F32, tag=f"sd{t_}", name=f"sd{t_}")
                                    for t_ in range(8)]
                            for it in range(IT):
                                w2t = shp.tile([128, 512], F32R, tag="ws2t")
                                nc.sync.dma_start(
                                    w2t[:],
                                    ws2c[it, :, hs * 512:(hs + 1) * 512])
                                for t_ in range(8):
                                    nc.tensor.matmul(
                                        pods[t_][:],
                                        lhsT=s_sb[:, it,
                                                  t_ * 128:(t_ + 1) * 128],
                                        rhs=w2t[:],
                                        start=(it == 0),
                                        stop=(it == IT - 1))
                            for t_ in range(8):
                                so = shp.tile([128, 512], F32, tag="so")
                                nc.vector.tensor_copy(so[:], pods[t_][:])
                                nc.sync.dma_start(
                                    shared_out[t0 + t_ * 128:
                                               t0 + (t_ + 1) * 128,
                                               hs * 512:(hs + 1) * 512],
                                    so[:])

            # ============ expert FFNs over gathered tokens ============
            with tc.tile_pool(name="ep", bufs=2) as ep:
                for e in range(EL):
                    xe = ep.tile([128, HB, C], F32R, tag="xe")
                    nc.sync.dma_start(xe[:], xe_dram[e])
                    a_sb = ep.tile([128, MT, C], F32R, tag="a_sb")
                    with tc.tile_pool(name="pse", bufs=1, space="PSUM") as pse:
                        for g0, g1 in MGROUPS:
                            pas = [pse.tile([128, C], F32, tag=f"pa{m - g0}", name=f"pa{m - g0}")
                                   for m in range(g0, g1)]
                            for hb in range(HB):
                                w1t = ep.tile([128, 512], F32R, tag="w1t")
                                nc.sync.dma_start(
                                    w1t[:, :(g1 - g0) * 128],
                                    w1l[e, hb, :, g0 * 128:g1 * 128])
                                for m in range(g0, g1):
                                    nc.tensor.matmul(
                                        pas[m - g0][:],
                                        lhsT=w1t[:, (m - g0) * 128:
                                                 (m - g0 + 1) * 128],
                                        rhs=xe[:, hb, :],
                                        start=(hb == 0), stop=(hb == HB - 1))
                            for m in range(g0, g1):
                                nc.scalar.activation(
                                    a_sb[:, m, :], pas[m - g0][:], AF.Silu)
                                nc.vector.tensor_mul(
                                    a_sb[:, m, :], a_sb[:, m, :],
                                    cwga_keep[e][:])
                    with tc.tile_pool(name="psf", bufs=1, space="PSUM") as psf:
                        for hs in range(4):
                            pos = [psf.tile([128, 512], F32, tag=f"po{st}", name=f"po{st}")
                                   for st in range(3)]
                            for m in range(MT):
                                w2t = ep.tile([128, 512], F32R, tag="w2t")
                                nc.sync.dma_start(
                                    w2t[:],
                                    w2l[e, m, :, hs * 512:(hs + 1) * 512])
                                for st in range(3):
                                    nc.tensor.matmul(
                                        pos[st][:],
                                        lhsT=a_sb[:, m,
                                                  st * 128:(st + 1) * 128],
                                        rhs=w2t[:],
                                        start=(m == 0), stop=(m == MT - 1))
                            for st in range(3):
                                so = ep.tile([128, 512], F32, tag="eso")
                                nc.vector.tensor_copy(so[:], pos[st][:])
                                nc.sync.dma_start(
                                    slots_out[e, st * 128:(st + 1) * 128,
                                              hs * 512:(hs + 1) * 512], so[:])
    nc.compile()
    return nc


def _stage_inputs(hidden_states, gate_w, w1, w2, ws1, ws2):
    x = np.ascontiguousarray(hidden_states.reshape(T, H), dtype=np.float32)
    xtb = np.ascontiguousarray(x.reshape(T, HB, 128).transpose(2, 0, 1))
    xtbr = np.ascontiguousarray(x.reshape(T, HB, 128).transpose(1, 2, 0))

    body = (np.arange(T, dtype=np.float32) + 1.0).reshape(TW, 16).T
    tail = (T + np.arange(C, dtype=np.float32) + 1.0).reshape(CW, 16).T
    iotap1 = np.ascontiguousarray(np.concatenate([body, tail], axis=1))

    in_maps = []
    perms = []
    for c in range(N_CORES):
        local = list(range(EL * c, EL * (c + 1)))
        rest = [e for e in range(E) if e not in local]
        perm = np.array(local + rest, dtype=np.int64)
        perms.append(perm)
        gwp = np.ascontiguousarray(gate_w[:, perm], dtype=np.float32)
        gwb = np.ascontiguousarray(gwp.reshape(HB, 128, E).transpose(1, 0, 2))
        w1c = np.ascontiguousarray(
            w1[local].reshape(EL, HB, 128, M), dtype=np.float32)
        w2c = np.ascontiguousarray(
            w2[local].reshape(EL, MT, 128, H), dtype=np.float32)
        ws1cc = np.ascontiguousarray(
            ws1[:, ISH * c:ISH * (c + 1)].reshape(HB, 128, ISH),
            dtype=np.float32)
        ws2cc = np.ascontiguousarray(
            ws2[ISH * c:ISH * (c + 1)].reshape(IT, 128, H), dtype=np.float32)
        in_maps.append({
            "xtb": xtb, "xtbr": xtbr, "gwb": gwb, "w1l": w1c, "w2l": w2c,
            "ws1c": ws1cc, "ws2c": ws2cc, "iotap1": iotap1,
        })
    return in_maps, perms


def run_cores(in_maps, **kwargs):
    if "nc" not in _CACHE:
        _CACHE["nc"] = _build_nc()
    return run_bass_kernel_spmd(
        _CACHE["nc"], in_maps, list(range(N_CORES)), **kwargs)


def combine(results, perms):
    out = np.zeros((T, H), dtype=np.float32)
    for c in range(N_CORES):
        out += results[c]["shared_out"]
    for c in range(N_CORES):
        slots = results[c]["slots_out"]          # [EL, C, H]
        toks = results[c]["tok_out"]             # [EL, 16, CW] fp32
        for e in range(EL):
            tok = toks[e].T.reshape(-1).astype(np.int64)  # unwrap
            valid = tok < T
            out[tok[valid]] += slots[e][valid]
    inv0 = np.argsort(perms[0])
    logits = results[0]["logits_out"][:, inv0]
    return (np.ascontiguousarray(out.reshape(B, S, H)),
            np.ascontiguousarray(logits.reshape(B, S, E)))


def kernel(**inputs):
    in_maps, perms = _stage_inputs(
        np.asarray(inputs["hidden_states"]), np.asarray(inputs["gate_w"]),
        np.asarray(inputs["w1"]), np.asarray(inputs["w2"]),
        np.asarray(inputs["ws1"]), np.asarray(inputs["ws2"]))
    res = run_cores(in_maps)
    return combine(res.results, perms)


# revision 10
# speedup vs baseline: 1.5234x; 1.5234x over previous
"""DeepSeek-MoE layer on 8 Trainium2 NeuronCores (expert-parallel, sparse dispatch).

Sharding:
  - Expert-parallel: core c owns experts [4c, 4c+4). The expert axis is
    permuted per core in the staged inputs so every core's LOCAL experts are
    logits columns 0..3 (keeps the SPMD program core-independent).
  - Shared expert: sharded on the intermediate dim (core c owns a 1024-wide
    slice of I=8192); partial [T, H] outputs summed on host.
  - Router + x replicated; router logits output taken from core 0.

Device program per core:
  1. Router logits [E, T] in true fp32 (top-4 selection is gap-sensitive:
     min 4th/5th logit gap in-distribution is ~5e-5, so reduced-precision
     matmuls would flip expert selections).
  2. Per 128-token tile: DVE max8 -> top-4 threshold + softmax combine
     weights cw in fp32 (matches jax top_k + softmax for distinct logits).
  3. Per local expert: GPSIMD sparse_gather compacts routed token ids
     (capacity C=384 >= observed max 281, tail-padded with out-of-range ids
     so padded slots self-identify); ap_gather pulls the tokens' hidden
     vectors from the resident x; bounced via DRAM.
  4. Shared-expert FFN over all tokens for the local I-slice.
  5. Expert FFNs over gathered tokens only. FFN matmuls run in fp16
     (fp32 PSUM accumulation; enables fast-weight-load, halves weight DMA);
     silu on ACT; per-slot combine weights applied in fp32 on the down-proj
     output via per-partition scalars.

Host: scatter-add slot outputs to token rows (the all-to-all return), sum
per-core shared partials, unpermute core 0's logits.
"""

import numpy as np

import concourse.bacc as bacc
import concourse.mybir as mybir
import concourse.tile as tile
from concourse.bass_utils import run_bass_kernel_spmd
from concourse.masks import make_identity

F32 = mybir.dt.float32
F16 = mybir.dt.float16
I16 = mybir.dt.int16
U32 = mybir.dt.uint32
AF = mybir.ActivationFunctionType
ALU = mybir.AluOpType
AXX = mybir.AxisListType.X

N_CORES = 8
B, S, H, E, K, M, I = 2, 1024, 2048, 32, 4, 1408, 8192
T = B * S                      # 2048 tokens
HB = H // 128                  # 16 h blocks
EL = E // N_CORES              # 4 local experts
MT = M // 128                  # 11 m tiles
ISH = I // N_CORES             # 1024 shared-intermediate slice
IT = ISH // 128                # 8 i tiles
C = 384                        # per-expert token capacity (max observed 281)
CW = C // 16
TW = T // 16                   # 128
NCH = 4                        # 512-token chunks
MGROUPS = ((0, 4), (4, 8), (8, 11))

_CACHE = {}


def _build_nc():
    nc = bacc.Bacc("TRN2", target_bir_lowering=False, debug=False,
                   num_devices=N_CORES)

    xtb = nc.dram_tensor("xtb", [128, T, HB], F16, kind="ExternalInput")
    xtbf = nc.dram_tensor("xtbf", [HB, 128, T], F32, kind="ExternalInput")
    gwb = nc.dram_tensor("gwb", [128, HB, E], F32, kind="ExternalInput")
    w1l = nc.dram_tensor("w1l", [EL, HB, 128, M], F16, kind="ExternalInput")
    w2l = nc.dram_tensor("w2l", [EL, MT, 128, H], F16, kind="ExternalInput")
    ws1c = nc.dram_tensor("ws1c", [HB, 128, ISH], F16, kind="ExternalInput")
    ws2c = nc.dram_tensor("ws2c", [IT, 128, H], F16, kind="ExternalInput")
    iotap1 = nc.dram_tensor("iotap1", [16, TW + CW], F32, kind="ExternalInput")

    logits_out = nc.dram_tensor("logits_out", [T, E], F32, kind="ExternalOutput")
    shared_out = nc.dram_tensor("shared_out", [T, H], F32, kind="ExternalOutput")
    slots_out = nc.dram_tensor("slots_out", [EL, C, H], F32, kind="ExternalOutput")
    tok_out = nc.dram_tensor("tok_out", [EL, 16, CW], F32, kind="ExternalOutput")

    cwT_dram = nc.dram_tensor("cwT_dram", [E, T], F32)
    xe_dram = nc.dram_tensor("xe_dram", [EL, 128, HB, C], F16)

    with tile.TileContext(nc) as tc:
        with tc.tile_pool(name="pp", bufs=1) as pp:
            ident = pp.tile([128, 128], F32)
            make_identity(nc, ident[:])
            iop1 = pp.tile([16, TW + CW], F32)
            nc.sync.dma_start(iop1[:], iotap1[:])
            cwl_keep = []

            # x in fp16, gather/shared layout — resident until experts begin
            with tc.tile_pool(name="xp", bufs=1) as xp:
                x_sb = xp.tile([128, T, HB], F16)
                for ch in range(8):
                    nc.sync.dma_start(x_sb[:, ch * 256:(ch + 1) * 256, :],
                                      xtb[:, ch * 256:(ch + 1) * 256, :])

                # ============ router + cw (true fp32, x streamed) ============
                with tc.tile_pool(name="rp", bufs=1) as rp, \
                     tc.tile_pool(name="rw", bufs=2) as rw, \
                     tc.tile_pool(name="rx", bufs=6) as rx, \
                     tc.tile_pool(name="ps1", bufs=1, space="PSUM") as ps1:
                    g_sb = rp.tile([128, HB, E], F32)
                    nc.sync.dma_start(g_sb[:], gwb[:])
                    lsb = rp.tile([E, T], F32)
                    for ch in range(NCH):
                        pt = ps1.tile([E, 512], F32, tag="rps")
                        for hb in range(HB):
                            xrt = rx.tile([128, 512], F32, tag="xrt")
                            nc.sync.dma_start(
                                xrt[:], xtbf[hb, :, ch * 512:(ch + 1) * 512])
                            nc.tensor.matmul(
                                pt[:], lhsT=g_sb[:, hb, :], rhs=xrt[:],
                                start=(hb == 0), stop=(hb == HB - 1))
                        nc.vector.tensor_copy(lsb[:, ch * 512:(ch + 1) * 512],
                                              pt[:])

                    cwT = rp.tile([E, T], F32)
                    for j in range(T // 128):
                        ptt = ps1.tile([128, E], F32, tag="tps")
                        nc.tensor.transpose(
                            ptt[:], lsb[:, j * 128:(j + 1) * 128], ident[:E, :E])
                        lf = rw.tile([128, E], F32, tag="lf")
                        nc.vector.tensor_copy(lf[:], ptt[:])
                        nc.sync.dma_start(logits_out[j * 128:(j + 1) * 128, :],
                                          lf[:])

                        m8 = rw.tile([128, 8], F32, tag="m8")
                        nc.vector.max(out=m8[:], in_=lf[:])
                        negmax = rw.tile([128, 1], F32, tag="negmax")
                        nc.vector.tensor_scalar_mul(negmax[:], m8[:, 0:1], -1.0)
                        exp4 = rw.tile([128, 4], F32, tag="exp4")
                        nc.scalar.activation(exp4[:], m8[:, 0:4], AF.Exp,
                                             bias=negmax[:])
                        den = rw.tile([128, 1], F32, tag="den")
                        nc.vector.reduce_sum(den[:], exp4[:], axis=AXX)
                        rden = rw.tile([128, 1], F32, tag="rden")
                        nc.vector.reciprocal(rden[:], den[:])

                        mask = rw.tile([128, E], F32, tag="mask")
                        nc.vector.tensor_tensor(
                            mask[:], lf[:], m8[:, 3:4].to_broadcast([128, E]),
                            op=ALU.is_ge)
                        cwt = rw.tile([128, E], F32, tag="cwt")
                        nc.scalar.activation(cwt[:], lf[:], AF.Exp,
                                             bias=negmax[:])
                        nc.vector.tensor_mul(cwt[:], cwt[:], mask[:])
                        nc.vector.tensor_scalar_mul(cwt[:], cwt[:], rden[:])

                        ptc = ps1.tile([E, 128], F32, tag="tpsb")
                        nc.tensor.transpose(ptc[:], cwt[:], ident[:])
                        nc.vector.tensor_copy(cwT[:, j * 128:(j + 1) * 128],
                                              ptc[:])
                    nc.sync.dma_start(cwT_dram[:], cwT[:])

                # ============ dispatch: tables + token gathers ============
                with tc.tile_pool(name="dw", bufs=2) as dw, \
                     tc.tile_pool(name="db", bufs=1) as db, \
                     tc.tile_pool(name="ps2", bufs=2, space="PSUM") as ps2:
                    cwl = pp.tile([128, EL, 3], F32, tag="cwl", name="cwl")
                    cwl_keep.append(cwl)
                    for e in range(EL):
                        v = dw.tile([128, 16], F32, tag="vrow")
                        nc.sync.dma_start(
                            v[:], cwT_dram[e].rearrange("(f q) -> f q", q=16))
                        wps = ps2.tile([16, 128], F32, tag="wps")
                        nc.tensor.transpose(wps[:], v[:], ident[:])
                        wext = dw.tile([16, TW + CW], F32, tag="wext")
                        nc.vector.tensor_copy(wext[:, :TW], wps[:])
                        nc.vector.memset(wext[:, TW:], 1.0)

                        ge0 = dw.tile([16, TW + CW], F32, tag="ge0")
                        nc.vector.tensor_scalar(ge0[:], wext[:], 0.0, None,
                                                op0=ALU.is_gt)
                        sgt = dw.tile([16, TW + CW], F32, tag="sgt")
                        nc.vector.tensor_mul(sgt[:], ge0[:], iop1[:])
                        nc.vector.tensor_scalar_add(sgt[:], sgt[:], -1.0)

                        tok_sl = dw.tile([16, CW], F32, tag="toksl")
                        nfound = dw.tile([1, 1], U32, tag="nf")
                        nc.gpsimd.sparse_gather(tok_sl[:], sgt[:],
                                                num_found=nfound[:])
                        nc.sync.dma_start(tok_out[e], tok_sl[:])

                        tok_cl = dw.tile([16, CW], F32, tag="tokcl")
                        nc.vector.tensor_scalar_min(tok_cl[:], tok_sl[:],
                                                    float(T - 1))
                        tok_i16 = dw.tile([16, CW], I16, tag="toki")
                        nc.vector.tensor_copy(tok_i16[:], tok_cl[:])
                        idx128 = dw.tile([128, CW], I16, tag="idx")
                        for k in range(8):
                            nc.sync.dma_start(idx128[k * 16:(k + 1) * 16, :],
                                              tok_i16[:])

                        # cw by slot: replicate cw row, gather, then extract
                        # slot-linear per-partition columns via PE transpose
                        cwrep = db.tile([128, T], F32, tag="cwrep")
                        nc.sync.dma_start(cwrep[0:1, :], cwT_dram[e][None, :])
                        p = 1
                        while p < 128:
                            nc.sync.dma_start(cwrep[p:2 * p, :], cwrep[0:p, :])
                            p *= 2
                        cwga = dw.tile([128, C], F32, tag="cwga")
                        nc.gpsimd.ap_gather(cwga[:], cwrep[:], idx128[:],
                                            channels=128, num_elems=T, d=1,
                                            num_idxs=C)
                        for st in range(3):
                            cps = ps2.tile([128, 128], F32, tag="cps")
                            nc.tensor.transpose(
                                cps[:], cwga[:, st * 128:(st + 1) * 128],
                                ident[:])
                            nc.vector.tensor_copy(cwl[:, e, st:st + 1],
                                                  cps[:, 0:1])

                        # token gather: 4 sub-gathers of 96 slots, repack
                        for g in range(4):
                            raw = db.tile([128, 96, HB], F16, tag="raw")
                            nc.gpsimd.ap_gather(
                                raw[:], x_sb[:], idx128[:, 6 * g:6 * g + 6],
                                channels=128, num_elems=T, d=HB, num_idxs=96)
                            xrp = db.tile([128, HB, 96], F16, tag="xrp")
                            for hb in range(HB):
                                nc.vector.tensor_copy(xrp[:, hb, :],
                                                      raw[:, :, hb])
                            nc.sync.dma_start(
                                xe_dram[e][:, :, 96 * g:96 * (g + 1)], xrp[:])

                # ============ shared expert (I-slice), fp16 ============
                with tc.tile_pool(name="shp", bufs=4) as shp, \
                     tc.tile_pool(name="shb", bufs=1) as shb:
                    s_sb = shb.tile([128, IT, T], F16)
                    with tc.tile_pool(name="psu", bufs=1, space="PSUM") as psu:
                        for half in range(2):
                            t0 = half * 1024
                            for g0 in (0, 4):
                                pts = [psu.tile([128, 512], F32,
                                                tag=f"su{q}", name=f"su{q}")
                                       for q in range(8)]
                                for hb in range(HB):
                                    wt = shp.tile([128, 512], F16, tag="ws1t")
                                    nc.sync.dma_start(
                                        wt[:],
                                        ws1c[hb, :, g0 * 128:(g0 + 4) * 128])
                                    for q in range(8):
                                        it, c2 = g0 + q // 2, q % 2
                                        nc.tensor.matmul(
                                            pts[q][:],
                                            lhsT=wt[:, (it - g0) * 128:
                                                    (it - g0 + 1) * 128],
                                            rhs=x_sb[:, t0 + c2 * 512:
                                                     t0 + (c2 + 1) * 512, hb],
                                            start=(hb == 0),
                                            stop=(hb == HB - 1))
                                for q in range(8):
                                    it, c2 = g0 + q // 2, q % 2
                                    nc.scalar.activation(
                                        s_sb[:, it, t0 + c2 * 512:
                                             t0 + (c2 + 1) * 512],
                                        pts[q][:], AF.Silu)
                    with tc.tile_pool(name="psd", bufs=1, space="PSUM") as psd:
                        for tg in range(2):
                            for hs in range(4):
                                pods = [psd.tile([128, 512], F32,
                                                 tag=f"sd{t_}", name=f"sd{t_}")
                                        for t_ in range(8)]
                                for it in range(IT):
                                    w2t = shp.tile([128, 512], F16, tag="ws2t")
                                    nc.sync.dma_start(
                                        w2t[:],
                                        ws2c[it, :, hs * 512:(hs + 1) * 512])
                                    for t_ in range(8):
                                        tt = tg * 8 + t_
                                        nc.tensor.matmul(
                                            pods[t_][:],
                                            lhsT=s_sb[:, it, tt * 128:
                                                      (tt + 1) * 128],
                                            rhs=w2t[:],
                                            start=(it == 0),
                                            stop=(it == IT - 1))
                                for t_ in range(8):
                                    tt = tg * 8 + t_
                                    so = shp.tile([128, 512], F32, tag="so")
                                    nc.vector.tensor_copy(so[:], pods[t_][:])
                                    nc.sync.dma_start(
                                        shared_out[tt * 128:(tt + 1) * 128,
                                                   hs * 512:(hs + 1) * 512],
                                        so[:])

            # ============ expert FFNs over gathered tokens (fp16) ============
            with tc.tile_pool(name="ep", bufs=2) as ep, \
                 tc.tile_pool(name="ew", bufs=6) as ew:
                cwl = cwl_keep[0]
                for e in range(EL):
                    xe = ep.tile([128, HB, C], F16, tag="xe")
                    nc.sync.dma_start(xe[:], xe_dram[e])
                    a_sb = ep.tile([128, MT, C], F16, tag="a_sb")
                    with tc.tile_pool(name="pse", bufs=1, space="PSUM") as pse:
                        for g0, g1 in MGROUPS:
                            pas = [pse.tile([128, C], F32, tag=f"pa{m - g0}",
                                            name=f"pa{m - g0}")
                                   for m in range(g0, g1)]
                            for hb in range(HB):
                                w1t = ew.tile([128, 512], F16, tag="w1t")
                                nc.sync.dma_start(
                                    w1t[:, :(g1 - g0) * 128],
                                    w1l[e, hb, :, g0 * 128:g1 * 128])
                                for m in range(g0, g1):
                                    nc.tensor.matmul(
                                        pas[m - g0][:],
                                        lhsT=w1t[:, (m - g0) * 128:
                                                 (m - g0 + 1) * 128],
                                        rhs=xe[:, hb, :],
                                        start=(hb == 0), stop=(hb == HB - 1))
                            for m in range(g0, g1):
                                nc.scalar.activation(
                                    a_sb[:, m, :], pas[m - g0][:], AF.Silu)
                    with tc.tile_pool(name="psf", bufs=1, space="PSUM") as psf:
                        for hs in range(4):
                            pos = [psf.tile([128, 512], F32, tag=f"po{st}",
                                            name=f"po{st}")
                                   for st in range(3)]
                            for m in range(MT):
                                w2t = ew.tile([128, 512], F16, tag="w2t")
                                nc.sync.dma_start(
                                    w2t[:],
                                    w2l[e, m, :, hs * 512:(hs + 1) * 512])
                                for st in range(3):
                                    nc.tensor.matmul(
                                        pos[st][:],
                                        lhsT=a_sb[:, m,
                                                  st * 128:(st + 1) * 128],
                                        rhs=w2t[:],
                                        start=(m == 0), stop=(m == MT - 1))
                            for st in range(3):
                                so = ew.tile([128, 512], F32, tag="eso")
                                nc.vector.tensor_scalar_mul(
                                    so[:], pos[st][:], cwl[:, e, st:st + 1])
                                nc.sync.dma_start(
                                    slots_out[e, st * 128:(st + 1) * 128,
                                              hs * 512:(hs + 1) * 512], so[:])
    nc.compile()
    return nc


def _stage_inputs(hidden_states, gate_w, w1, w2, ws1, ws2):
    x = np.ascontiguousarray(hidden_states.reshape(T, H), dtype=np.float32)
    xtb = np.ascontiguousarray(
        x.reshape(T, HB, 128).transpose(2, 0, 1).astype(np.float16))
    xtbf = np.ascontiguousarray(x.reshape(T, HB, 128).transpose(1, 2, 0))

    body = (np.arange(T, dtype=np.float32) + 1.0).reshape(TW, 16).T
    tail = (T + np.arange(C, dtype=np.float32) + 1.0).reshape(CW, 16).T
    iotap1 = np.ascontiguousarray(np.concatenate([body, tail], axis=1))

    in_maps = []
    perms = []
    for c in range(N_CORES):
        local = list(range(EL * c, EL * (c + 1)))
        rest = [e for e in range(E) if e not in local]
        perm = np.array(local + rest, dtype=np.int64)
        perms.append(perm)
        gwp = np.ascontiguousarray(gate_w[:, perm], dtype=np.float32)
        gwb = np.ascontiguousarray(gwp.reshape(HB, 128, E).transpose(1, 0, 2))
        w1c = np.ascontiguousarray(
            w1[local].reshape(EL, HB, 128, M).astype(np.float16))
        w2c = np.ascontiguousarray(
            w2[local].reshape(EL, MT, 128, H).astype(np.float16))
        ws1cc = np.ascontiguousarray(
            ws1[:, ISH * c:ISH * (c + 1)].reshape(HB, 128, ISH)
            .astype(np.float16))
        ws2cc = np.ascontiguousarray(
            ws2[ISH * c:ISH * (c + 1)].reshape(IT, 128, H).astype(np.float16))
        in_maps.append({
            "xtb": xtb, "xtbf": xtbf, "gwb": gwb, "w1l": w1c, "w2l": w2c,
            "ws1c": ws1cc, "ws2c": ws2cc, "iotap1": iotap1,
        })
    return in_maps, perms


def run_cores(in_maps, **kwargs):
    if "nc" not in _CACHE:
        _CACHE["nc"] = _build_nc()
    return run_bass_kernel_spmd(
        _CACHE["nc"], in_maps, list(range(N_CORES)), **kwargs)


def combine(results, perms):
    out = np.zeros((T, H), dtype=np.float32)
    for c in range(N_CORES):
        out += results[c]["shared_out"]
    for c in range(N_CORES):
        slots = results[c]["slots_out"]          # [EL, C, H]
        toks = results[c]["tok_out"]             # [EL, 16, CW] fp32
        for e in range(EL):
            tok = toks[e].T.reshape(-1).astype(np.int64)  # unwrap
            valid = tok < T
            out[tok[valid]] += slots[e][valid]
    inv0 = np.argsort(perms[0])
    logits = results[0]["logits_out"][:, inv0]
    return (np.ascontiguousarray(out.reshape(B, S, H)),
            np.ascontiguousarray(logits.reshape(B, S, E)))


def kernel(**inputs):
    in_maps, perms = _stage_inputs(
        np.asarray(inputs["hidden_states"]), np.asarray(inputs["gate_w"]),
        np.asarray(inputs["w1"]), np.asarray(inputs["w2"]),
        np.asarray(inputs["ws1"]), np.asarray(inputs["ws2"]))
    res = run_cores(in_maps)
    return combine(res.results, perms)
